# revision 1
# baseline (speedup 1.0000x reference)
"""Trainium2 Bass kernel for nn_BaseLineModel (segment_reduce).

Model: token-embed -> conv1d(K=3) -> relu -> global max-pool per note,
concat with delta-times, segment-mean over notes sharing a start day,
tiny linear + sigmoid -> [S, 1].

Sharding: notes (N=16384) split 8 ways. Per core: gather embeddings for
2048 notes (dma_gather transpose mode, bf16, rows padded to 256B), conv
as 2 PSUM-accumulated matmuls (contraction (e,k0|k1)=128 via an SBUF
shifted-copy stack, plus (e,k2)=64 as an AP view), DVE max-pool, ACT
relu+bias, PE transpose to note-major, segment-sum over S=1024 days via
one-hot float32r matmuls, ReduceScatter(add) across the 8 cores, and the
final mean/linear/sigmoid on each core's 128-day slice.
"""

import numpy as np
import ml_dtypes

import concourse.bass as bass
import concourse.mybir as mybir
import concourse.tile as tile
from concourse.bass_utils import run_bass_kernel_spmd
from concourse import library_config
from concourse.masks import make_identity
from concourse.vector_clock import ScopedClock  # noqa: F401  (import sanity)

# ---- problem dims (hardcoded per task contract) ----
N, L, E, H, K, V, S = 16384, 64, 64, 256, 3, 30000, 1024
NCORES = 8
NC_NOTES = N // NCORES            # 2048 notes per core
NTOK = NC_NOTES * L               # 131072 tokens per core
BLK_NOTES = 128                   # notes per gather block
BLK_TOK = BLK_NOTES * L           # 8192 tokens per block
NBLK = NC_NOTES // BLK_NOTES      # 16
GRP_NOTES = 8                     # notes per matmul group (512 cols)
NGRP = BLK_NOTES // GRP_NOTES     # 16 groups per block
GCHUNK = 512                      # tokens per dma_gather call (desc-ring cap)
NGC = BLK_TOK // GCHUNK           # gather calls per block
NCHUNK = NC_NOTES // 128          # 16 note-chunks for segment phase
F = 258                           # main cols: delta | 256 feats | ones
TMAX = L - K + 1                  # 62 valid conv positions

_SPLIT_MAXW = 1


def _split_waits(nc, maxw=_SPLIT_MAXW):
    """This walrus build rejects >1 sync wait per instruction; move extras
    onto preceding same-engine NOPs (sequencer order preserves semantics)."""
    for bb in nc.main_func.blocks:
        out = []
        for inst in bb.instructions:
            si = inst.sync_info
            waits = list(si.on_wait) if (si is not None and si.on_wait) else []
            if len(waits) > maxw:
                rest = waits[:-maxw]
                si.on_wait = waits[-maxw:]
                for i in range(0, len(rest), maxw):
                    out.append(mybir.InstNoOp(
                        name=f"{inst.name}-wsplit{i}",
                        sync_info=mybir.SyncInfo(on_wait=rest[i:i + maxw], on_update=[]),
                        bass_nofuse=True,
                        engine=inst.engine,
                    ))
            out.append(inst)
        bb.instructions = out


def _build_nc(reps=1, use_cc=True, mode='full'):
    f32 = mybir.dt.float32
    f32r = mybir.dt.float32r
    bf16 = mybir.dt.bfloat16
    i16 = mybir.dt.int16

    nc = bass.Bass()
    d_embp = nc.declare_dram_parameter("embp", [V, 128], bf16, isOutput=False)
    d_idx = nc.declare_dram_parameter("idx", [NBLK, 128, BLK_TOK // 16], i16, isOutput=False)
    d_stf = nc.declare_dram_parameter("stf", [NC_NOTES, 1], f32, isOutput=False)
    d_delta = nc.declare_dram_parameter("delta", [NC_NOTES, 1], f32, isOutput=False)
    d_w01 = nc.declare_dram_parameter("w01", [128, H], bf16, isOutput=False)
    d_w2 = nc.declare_dram_parameter("w2", [64, H], bf16, isOutput=False)
    d_cb = nc.declare_dram_parameter("convb2", [128, 2], f32, isOutput=False)
    d_iota = nc.declare_dram_parameter("iota", [128, S], f32, isOutput=False)
    d_wrep = nc.declare_dram_parameter("wrep", [128, H + 1], f32, isOutput=False)
    d_brep = nc.declare_dram_parameter("brep", [128, 1], f32, isOutput=False)
    d_out = nc.declare_dram_parameter("out", [128, 1], f32, isOutput=True)
    part = nc.dram_tensor("part", [S, F], f32)
    rs_out = nc.dram_tensor("rs_out", [128, F], f32)

    with tile.TileContext(nc) as tc:
        nc.gpsimd.load_library(library_config.mlp)
        nidx_reg_cm = nc.gpsimd.register("nidx")
        nidx_reg = nidx_reg_cm.__enter__()
        nc.gpsimd.reg_mov(nidx_reg, GCHUNK)
        rep_range = range(reps)
        with (
            tc.tile_pool(name="cst", bufs=1) as cp,
            tc.tile_pool(name="feat", bufs=1) as fp,
            tc.tile_pool(name="mainp", bufs=1) as mp,
        ):
         for _rep in rep_range:
             w01_sb = cp.tile([128, H], bf16)
             w2_sb = cp.tile([64, H], bf16)
             cb_sb = cp.tile([128, 2], f32)
             ident = cp.tile([128, 128], f32)
             nc.sync.dma_start(out=w01_sb[:], in_=d_w01[:])
             nc.sync.dma_start(out=w2_sb[:], in_=d_w2[:])
             nc.sync.dma_start(out=cb_sb[:], in_=d_cb[:])
             make_identity(nc, ident[:])
             feats = [fp.tile([128, NC_NOTES], f32, name=f"feats{hh}") for hh in range(2)]

             # ---- P1: gather + conv + maxpool, per block ----
             with (
                 tc.tile_pool(name="gath", bufs=3) as gp,
                 tc.tile_pool(name="ypsum", bufs=6, space="PSUM") as yp,
             ):
                 for b in range(NBLK):
                     idx_sb = gp.tile([128, BLK_TOK // 16], i16, tag="idx")
                     nc.sync.dma_start(out=idx_sb[:], in_=d_idx[b])
                     x_sb = gp.tile([128, BLK_TOK], bf16, tag="x")
                     for c in range(NGC):
                         nc.gpsimd.dma_gather(
                             out_ap=x_sb[:, c * GCHUNK:(c + 1) * GCHUNK]
                                 .rearrange("p (o n) -> p o n", o=1),
                             in_ap=d_embp[:],
                             idxs_ap=idx_sb[:, c * (GCHUNK // 16):(c + 1) * (GCHUNK // 16)],
                             num_idxs=GCHUNK,
                             num_idxs_reg=nidx_reg,
                             elem_size=128,
                             transpose=True,
                         )
                     # stack k=1 shift into partitions 64:128 (SBUF->SBUF DMA)
                     nc.sync.dma_start(out=x_sb[64:128, 0:BLK_TOK - 1],
                                       in_=x_sb[0:64, 1:BLK_TOK])
                     for g in (range(NGRP) if mode != 'gather' else ()):
                         c0 = g * 512
                         for hh in range(2):
                             y_ps = yp.tile([128, 512], f32, tag="y", name=f"y{b}_{g}_{hh}")
                             nc.tensor.matmul(out=y_ps[:],
                                              lhsT=w01_sb[:, hh * 128:(hh + 1) * 128],
                                              rhs=x_sb[:, c0:c0 + 512],
                                              start=True, stop=False)
                             nc.tensor.matmul(out=y_ps[:, 0:510],
                                              lhsT=w2_sb[:, hh * 128:(hh + 1) * 128],
                                              rhs=x_sb[0:64, c0 + 2:c0 + 512],
                                              start=False, stop=True)
                             nc.vector.reduce_max(
                                 out=feats[hh][:, b * BLK_NOTES + g * 8:
                                               b * BLK_NOTES + g * 8 + 8],
                                 in_=y_ps[:].rearrange("p (n l) -> p n l", l=L)[:, :, 0:TMAX],
                                 axis=mybir.AxisListType.X)

             # ---- P2: relu(feats + conv_b) ----
             if mode == 'gather':
                 nc.vector.memset(feats[0][:], 0.0)
                 nc.vector.memset(feats[1][:], 0.0)
             for hh in range(2):
                 nc.scalar.activation(out=feats[hh][:], in_=feats[hh][:],
                                      func=mybir.ActivationFunctionType.Relu,
                                      bias=cb_sb[:, hh:hh + 1], scale=1.0)

             # ---- P3: transpose to note-major main tiles ----
             mains = [mp.tile([128, F], f32, name=f"main{i}") for i in range(NCHUNK)]
             with tc.tile_pool(name="tpsum", bufs=2, space="PSUM") as tp:
                 for i in range(NCHUNK):
                     nc.sync.dma_start(out=mains[i][:, 0:1],
                                       in_=d_delta[i * 128:(i + 1) * 128, :])
                     nc.vector.memset(mains[i][:, H + 1:H + 2], 1.0)
                     for hh in range(2):
                         t_ps = tp.tile([128, 128], f32, tag="t", name=f"t{i}_{hh}")
                         nc.tensor.transpose(out=t_ps[:],
                                             in_=feats[hh][:, i * 128:(i + 1) * 128],
                                             identity=ident[:])
                         nc.vector.tensor_copy(
                             out=mains[i][:, 1 + hh * 128:1 + (hh + 1) * 128],
                             in_=t_ps[:])

             # ---- P4: segment-sum via one-hot matmuls ----
             with (
                 tc.tile_pool(name="segsb", bufs=2) as ssp,
                 tc.tile_pool(name="segps", bufs=1, space="PSUM") as pp,
             ):
                 iota_sb = cp.tile([128, S], f32)
                 nc.sync.dma_start(out=iota_sb[:], in_=d_iota[:])
                 seg_ps = [pp.tile([128, F], f32, tag=f"seg{bk}", name=f"seg{bk}")
                           for bk in range(8)]
                 for i in range(NCHUNK):
                     st_sb = ssp.tile([128, 1], f32, tag="st")
                     nc.sync.dma_start(out=st_sb[:], in_=d_stf[i * 128:(i + 1) * 128, :])
                     oh_sb = ssp.tile([128, S], f32, tag="oh")
                     nc.vector.tensor_tensor(out=oh_sb[:],
                                             in0=st_sb[:, 0:1].to_broadcast([128, S]),
                                             in1=iota_sb[:],
                                             op=mybir.AluOpType.is_equal)
                     for bk in range(8):
                         nc.tensor.matmul(out=seg_ps[bk][:],
                                          lhsT=oh_sb[:, bk * 128:(bk + 1) * 128],
                                          rhs=mains[i][:],
                                          start=(i == 0), stop=(i == NCHUNK - 1))
                 for bk in range(8):
                     seg_sb = ssp.tile([128, F], f32, tag="segout")
                     nc.vector.tensor_copy(out=seg_sb[:], in_=seg_ps[bk][:])
                     nc.sync.dma_start(out=part[bk * 128:(bk + 1) * 128, :], in_=seg_sb[:])

             # ---- P5: cross-core reduce + finalize ----
             if use_cc:
                 with tc.tile_critical():
                     with nc.semaphore("cc_sem") as cc_sem:
                         nc.gpsimd.collective_compute(
                             "ReduceScatter", mybir.AluOpType.add,
                             replica_groups=[list(range(NCORES))],
                             ins=[part[:]], outs=[rs_out[:]],
                         ).then_inc(cc_sem, 1)
                         nc.gpsimd.wait_ge(cc_sem, 1)
             else:
                 nc.sync.dma_start(out=rs_out[:], in_=part[0:128, :])

             with tc.tile_pool(name="fin", bufs=1) as fin:
                 wrep_sb = fin.tile([128, H + 1], f32)
                 nc.sync.dma_start(out=wrep_sb[:], in_=d_wrep[:])
                 brep_sb = fin.tile([128, 1], f32)
                 nc.sync.dma_start(out=brep_sb[:], in_=d_brep[:])
                 fs = fin.tile([128, F], f32)
                 nc.sync.dma_start(out=fs[:], in_=rs_out[:])
                 cnt = fin.tile([128, 1], f32)
                 nc.vector.tensor_scalar_max(out=cnt[:], in0=fs[:, H + 1:H + 2], scalar1=1.0)
                 rcp = fin.tile([128, 1], f32)
                 nc.vector.reciprocal(out=rcp[:], in_=cnt[:])
                 prod = fin.tile([128, H + 1], f32)
                 nc.vector.tensor_tensor(out=prod[:], in0=fs[:, 0:H + 1], in1=wrep_sb[:],
                                         op=mybir.AluOpType.mult)
                 dot = fin.tile([128, 1], f32)
                 nc.vector.reduce_sum(out=dot[:], in_=prod[:], axis=mybir.AxisListType.X)
                 nc.vector.tensor_scalar_mul(out=dot[:], in0=dot[:], scalar1=rcp[:, 0:1])
                 nc.vector.tensor_add(out=dot[:], in0=dot[:], in1=brep_sb[:])
                 outsb = fin.tile([128, 1], f32)
                 nc.scalar.activation(out=outsb[:], in_=dot[:],
                                      func=mybir.ActivationFunctionType.Sigmoid, scale=1.0)
                 nc.sync.dma_start(out=d_out[:], in_=outsb[:])

    _split_waits(nc)
    mybir.codegen_inst_isa_subclasses(nc)
    return nc


_NC_CACHE = {}


def _get_nc(reps=1, use_cc=True, mode='full'):
    key = (reps, use_cc, mode)
    if key not in _NC_CACHE:
        _NC_CACHE[key] = _build_nc(reps, use_cc, mode)
    return _NC_CACHE[key]


def _prep_inputs(text, start_times, emb, conv_w, conv_b, W, b):
    bf16 = ml_dtypes.bfloat16
    text = np.asarray(text)[0]              # [N, L]
    st = np.asarray(start_times)[0].astype(np.int64)   # [N]
    emb = np.asarray(emb, dtype=np.float32)
    conv_w = np.asarray(conv_w, dtype=np.float32)
    conv_b = np.asarray(conv_b, dtype=np.float32)
    W = np.asarray(W, dtype=np.float32)
    b = np.asarray(b, dtype=np.float32)

    embp = np.zeros((V, 128), dtype=bf16)
    embp[:, :E] = emb.astype(bf16)

    w01 = np.zeros((128, H), dtype=bf16)
    w01[:64, :] = conv_w[:, :, 0].T.astype(bf16)
    w01[64:, :] = conv_w[:, :, 1].T.astype(bf16)
    w2 = np.ascontiguousarray(conv_w[:, :, 2].T.astype(bf16))
    convb2 = np.ascontiguousarray(conv_b.reshape(2, 128).T.astype(np.float32))

    iota = np.tile(np.arange(S, dtype=np.float32), (128, 1))
    wrep = np.tile(W[:H + 1, 0], (128, 1)).astype(np.float32)
    brep = np.full((128, 1), b[0], np.float32)

    delta_g = np.concatenate([[0.0], np.diff(st).astype(np.float32)]).astype(np.float32)

    tok = text.astype(np.int16)             # V=30000 < 2**15
    in_maps = []
    for c in range(NCORES):
        sl = slice(c * NC_NOTES, (c + 1) * NC_NOTES)
        t = tok[sl].reshape(NBLK, BLK_TOK // GCHUNK, GCHUNK)
        # per-chunk wrap: [32, 16] -> [16, 32], tiled to 128 partitions
        w = t.reshape(NBLK, BLK_TOK // GCHUNK, GCHUNK // 16, 16)
        w = w.transpose(0, 1, 3, 2)                 # [NBLK, NGC, 16, GCHUNK//16]
        w = np.tile(w, (1, 1, 8, 1))                # [NBLK, NGC, 128, GCHUNK//16]
        idx = np.ascontiguousarray(
            w.transpose(0, 2, 1, 3).reshape(NBLK, 128, BLK_TOK // 16))
        in_maps.append({
            "embp": embp,
            "idx": idx,
            "stf": np.ascontiguousarray(st[sl, None].astype(np.float32)),
            "delta": np.ascontiguousarray(delta_g[sl, None]),
            "w01": w01,
            "w2": w2,
            "convb2": convb2,
            "iota": iota,
            "wrep": wrep,
            "brep": brep,
        })
    return in_maps


def kernel(**inputs) -> np.ndarray:
    nc = _get_nc()
    in_maps = _prep_inputs(**inputs)
    res = run_bass_kernel_spmd(nc, in_maps, list(range(NCORES))).results
    out = np.concatenate([res[c]["out"] for c in range(NCORES)], axis=0)
    return out.astype(np.float32)


if __name__ == "__main__":
    import jax
    import reference
    cpu = jax.devices("cpu")[0]
    with jax.default_device(cpu):
        ins = {k: np.asarray(v) for k, v in reference.setup_inputs().items()}
        exp = np.asarray(reference.reference(**reference.setup_inputs()))
    got = kernel(**ins)
    err = np.abs(got - exp).max()
    rel = err / max(np.abs(exp).max(), 1e-9)
    print("max abs err:", err, "rel:", rel)



# revision 5
# speedup vs baseline: 1.2738x; 1.2738x over previous
"""Trainium2 Bass kernel for nn_BaseLineModel (segment_reduce).

Model: token-embed -> conv1d(K=3) -> relu -> global max-pool per note,
concat with delta-times, segment-mean over notes sharing a start day,
tiny linear + sigmoid -> [S, 1].

Since the final Linear is applied to the segment-MEAN, and mean/sum
commute with the linear map, we collapse per-note features to a single
scalar z[n] = feats[n]·W[1:] + delta[n]*W[0] on-device, segment-sum
[z, 1] per day ([S, 2] instead of [S, 258]) and finish with
sigmoid(z_s/max(c_s,1) + b) after an 8-core ReduceScatter.

Sharding: notes (N=16384) split 8 ways. Per core: token embeddings are
fetched with non-transpose dma_gather (one 256B descriptor per token,
round-robin over 4 SWDGE queues - measured ~20x faster than the
transpose-mode gather), PE-transposed from token-major to E-major,
conv as 2 PSUM-accumulated matmuls per 512 tokens per 128-h half,
DVE max-pool, ACT relu+bias, per-note z via PE transpose + DVE dot,
day one-hot matmul segment-sum, ReduceScatter(add) on [1024, 2].
"""

import numpy as np
import ml_dtypes

import concourse.bass as bass
import concourse.mybir as mybir
import concourse.tile as tile
from concourse.bass_utils import run_bass_kernel_spmd
from concourse import library_config
from concourse.masks import make_identity

# ---- problem dims (hardcoded per task contract) ----
N, L, E, H, K, V, S = 16384, 64, 64, 256, 3, 30000, 1024
NCORES = 8
NC_NOTES = N // NCORES            # 2048 notes per core
NTOK = NC_NOTES * L               # 131072 tokens per core
BLK_NOTES = 128                   # notes per gather block
BLK_TOK = BLK_NOTES * L           # 8192 tokens per block
NBLK = NC_NOTES // BLK_NOTES      # 16
GCHUNK = 512                      # tokens per dma_gather call (desc-ring cap)
NGC = BLK_TOK // GCHUNK           # 16 gather calls per block
NQ = 4                            # SWDGE queues (ucode max)
NCHUNK = NC_NOTES // 128          # 16 note-chunks for segment phase
TMAX = L - K + 1                  # 62 valid conv positions

_SPLIT_MAXW = 1


def _split_waits(nc, maxw=_SPLIT_MAXW):
    """This walrus build rejects >1 sync wait per instruction; move extras
    onto preceding same-engine NOPs (sequencer order preserves semantics)."""
    for bb in nc.main_func.blocks:
        out = []
        for inst in bb.instructions:
            si = inst.sync_info
            waits = list(si.on_wait) if (si is not None and si.on_wait) else []
            if len(waits) > maxw:
                rest = waits[:-maxw]
                si.on_wait = waits[-maxw:]
                for i in range(0, len(rest), maxw):
                    out.append(mybir.InstNoOp(
                        name=f"{inst.name}-wsplit{i}",
                        sync_info=mybir.SyncInfo(on_wait=rest[i:i + maxw], on_update=[]),
                        bass_nofuse=True,
                        engine=inst.engine,
                    ))
            out.append(inst)
        bb.instructions = out


def _build_nc(reps=1, use_cc=True, mode='full'):
    f32 = mybir.dt.float32
    bf16 = mybir.dt.bfloat16
    i16 = mybir.dt.int16

    nc = bass.Bass(num_swdge_queues=NQ)
    d_embp = nc.declare_dram_parameter("embp", [V, 128], bf16, isOutput=False)
    d_idx = nc.declare_dram_parameter("idx", [NBLK, 128, BLK_TOK // 16], i16, isOutput=False)
    d_stf = nc.declare_dram_parameter("stf", [NC_NOTES, 1], f32, isOutput=False)
    d_dw0 = nc.declare_dram_parameter("dw0", [NC_NOTES, 1], f32, isOutput=False)
    d_w01 = nc.declare_dram_parameter("w01", [128, H], bf16, isOutput=False)
    d_w2 = nc.declare_dram_parameter("w2", [64, H], bf16, isOutput=False)
    d_cb = nc.declare_dram_parameter("convb2", [128, 2], f32, isOutput=False)
    d_iota = nc.declare_dram_parameter("iota", [128, S], f32, isOutput=False)
    d_wfeat = nc.declare_dram_parameter("wfeat", [128, H], f32, isOutput=False)
    d_brep = nc.declare_dram_parameter("brep", [128, 1], f32, isOutput=False)
    d_out = nc.declare_dram_parameter("out", [128, 1], f32, isOutput=True)
    part = nc.dram_tensor("part", [S, 2], f32)
    rs_out = nc.dram_tensor("rs_out", [128, 2], f32)

    with tile.TileContext(nc) as tc:
        nc.gpsimd.load_library(library_config.mlp)
        nidx_reg_cm = nc.gpsimd.register("nidx")
        nidx_reg = nidx_reg_cm.__enter__()
        nc.gpsimd.reg_mov(nidx_reg, GCHUNK)
        with (
            tc.tile_pool(name="cst", bufs=1) as cp,
            tc.tile_pool(name="feat", bufs=1) as fp,
        ):
         for _rep in range(reps):
             w01_sb = cp.tile([128, H], bf16)
             w2_sb = cp.tile([64, H], bf16)
             cb_sb = cp.tile([128, 2], f32)
             identb = cp.tile([128, 128], bf16)
             identf = cp.tile([128, 128], f32)
             nc.sync.dma_start(out=w01_sb[:], in_=d_w01[:])
             nc.sync.dma_start(out=w2_sb[:], in_=d_w2[:])
             nc.sync.dma_start(out=cb_sb[:], in_=d_cb[:])
             make_identity(nc, identb[:])
             make_identity(nc, identf[:])
             feats = [fp.tile([128, NC_NOTES], f32, name=f"feats{hh}") for hh in range(2)]

             # ---- P1: gather (token-major) + PE transpose + conv + maxpool ----
             with (
                 tc.tile_pool(name="gath", bufs=3) as gp,
                 tc.tile_pool(name="xep", bufs=2) as xp,
                 tc.tile_pool(name="tpsum", bufs=2, space="PSUM") as tp,
                 tc.tile_pool(name="ypsum", bufs=4, space="PSUM") as yp,
             ):
                 for b in range(NBLK):
                     idx_sb = gp.tile([128, BLK_TOK // 16], i16, tag="idx")
                     nc.sync.dma_start(out=idx_sb[:], in_=d_idx[b])
                     stg = gp.tile([128, BLK_TOK], bf16, tag="stg")
                     for c in range(NGC):
                         nc.gpsimd.dma_gather(
                             out_ap=stg[:, c * GCHUNK:(c + 1) * GCHUNK]
                                 .rearrange("p (g e) -> p g e", e=128),
                             in_ap=d_embp[:],
                             idxs_ap=idx_sb[:, c * (GCHUNK // 16):(c + 1) * (GCHUNK // 16)],
                             num_idxs=GCHUNK,
                             num_idxs_reg=nidx_reg,
                             elem_size=128,
                             transpose=False,
                             queue_num=c % NQ,
                         )
                     xE = xp.tile([128, BLK_TOK], bf16, tag="xe")
                     if mode != 'gather':
                         # token-major [128 tok, 128 e] tiles -> E-major xE
                         for q in range(BLK_TOK // 512):
                             tq = tp.tile([128, 512], bf16, tag="t", name=f"t{b}_{q}")
                             for j in range(4):
                                 g = q * 4 + j
                                 nc.tensor.transpose(
                                     out=tq[:, j * 128:(j + 1) * 128],
                                     in_=stg[:, g * 128:(g + 1) * 128],
                                     identity=identb[:])
                             nc.scalar.activation(
                                 out=xE[0:64, q * 512:(q + 1) * 512],
                                 in_=tq[0:64, :],
                                 func=mybir.ActivationFunctionType.Copy,
                                 scale=1.0)
                         # stack k=1 shift into partitions 64:128
                         nc.sync.dma_start(out=xE[64:128, 0:BLK_TOK - 1],
                                           in_=xE[0:64, 1:BLK_TOK])
                         for g in range(16):
                             c0 = g * 512
                             for hh in range(2):
                                 y_ps = yp.tile([128, 512], f32, tag="y", name=f"y{b}_{g}_{hh}")
                                 nc.tensor.matmul(out=y_ps[:],
                                                  lhsT=w01_sb[:, hh * 128:(hh + 1) * 128],
                                                  rhs=xE[:, c0:c0 + 512],
                                                  start=True, stop=False)
                                 nc.tensor.matmul(out=y_ps[:, 0:510],
                                                  lhsT=w2_sb[:, hh * 128:(hh + 1) * 128],
                                                  rhs=xE[0:64, c0 + 2:c0 + 512],
                                                  start=False, stop=True)
                                 nc.vector.reduce_max(
                                     out=feats[hh][:, b * BLK_NOTES + g * 8:
                                                   b * BLK_NOTES + g * 8 + 8],
                                     in_=y_ps[:].rearrange("p (n l) -> p n l", l=L)[:, :, 0:TMAX],
                                     axis=mybir.AxisListType.X)

             # ---- P2: relu(feats + conv_b) ----
             if mode == 'gather':
                 nc.vector.memset(feats[0][:], 0.0)
                 nc.vector.memset(feats[1][:], 0.0)
             for hh in range(2):
                 nc.scalar.activation(out=feats[hh][:], in_=feats[hh][:],
                                      func=mybir.ActivationFunctionType.Relu,
                                      bias=cb_sb[:, hh:hh + 1], scale=1.0)

             # ---- P3: per-note z = feats.W[1:] + delta*W[0]; day one-hot segsum ----
             wfeat_sb = cp.tile([128, H], f32)
             nc.sync.dma_start(out=wfeat_sb[:], in_=d_wfeat[:])
             iota_sb = cp.tile([128, S], f32)
             nc.sync.dma_start(out=iota_sb[:], in_=d_iota[:])
             with (
                 tc.tile_pool(name="segsb", bufs=2) as ssp,
                 tc.tile_pool(name="ftps", bufs=2, space="PSUM") as ftp,
                 tc.tile_pool(name="segps", bufs=1, space="PSUM") as pp,
             ):
                 seg_ps = pp.tile([128, 16], f32, name="seg")
                 for i in range(NCHUNK):
                     ft = ftp.tile([128, H], f32, tag="ft", name=f"ft{i}")
                     for hh in range(2):
                         nc.tensor.transpose(
                             out=ft[:, hh * 128:(hh + 1) * 128],
                             in_=feats[hh][:, i * 128:(i + 1) * 128],
                             identity=identf[:])
                     prod = ssp.tile([128, H], f32, tag="prod")
                     nc.vector.tensor_tensor(out=prod[:], in0=ft[:], in1=wfeat_sb[:],
                                             op=mybir.AluOpType.mult)
                     dw0 = ssp.tile([128, 1], f32, tag="dw0")
                     nc.sync.dma_start(out=dw0[:], in_=d_dw0[i * 128:(i + 1) * 128, :])
                     z2 = ssp.tile([128, 2], f32, tag="z2")
                     nc.vector.reduce_sum(out=z2[:, 0:1], in_=prod[:],
                                          axis=mybir.AxisListType.X)
                     nc.vector.tensor_add(out=z2[:, 0:1], in0=z2[:, 0:1], in1=dw0[:])
                     nc.vector.memset(z2[:, 1:2], 1.0)
                     st_sb = ssp.tile([128, 1], f32, tag="st")
                     nc.sync.dma_start(out=st_sb[:], in_=d_stf[i * 128:(i + 1) * 128, :])
                     oh_sb = ssp.tile([128, S], f32, tag="oh")
                     nc.vector.tensor_tensor(out=oh_sb[:],
                                             in0=st_sb[:, 0:1].to_broadcast([128, S]),
                                             in1=iota_sb[:],
                                             op=mybir.AluOpType.is_equal)
                     for bk in range(8):
                         nc.tensor.matmul(out=seg_ps[:, bk * 2:bk * 2 + 2],
                                          lhsT=oh_sb[:, bk * 128:(bk + 1) * 128],
                                          rhs=z2[:],
                                          start=(i == 0), stop=(i == NCHUNK - 1))
                 seg_sb = ssp.tile([128, 16], f32, tag="segout")
                 nc.vector.tensor_copy(out=seg_sb[:], in_=seg_ps[:])
                 for bk in range(8):
                     nc.sync.dma_start(out=part[bk * 128:(bk + 1) * 128, :],
                                       in_=seg_sb[:, bk * 2:bk * 2 + 2])

             # ---- P4: cross-core reduce + finalize ----
             if use_cc:
                 with tc.tile_critical():
                     with nc.semaphore("cc_sem") as cc_sem:
                         nc.gpsimd.collective_compute(
                             "ReduceScatter", mybir.AluOpType.add,
                             replica_groups=[list(range(NCORES))],
                             ins=[part[:]], outs=[rs_out[:]],
                         ).then_inc(cc_sem, 1)
                         nc.gpsimd.wait_ge(cc_sem, 1)
             else:
                 nc.sync.dma_start(out=rs_out[:], in_=part[0:128, :])

             with tc.tile_pool(name="fin", bufs=1) as fin:
                 brep_sb = fin.tile([128, 1], f32)
                 nc.sync.dma_start(out=brep_sb[:], in_=d_brep[:])
                 fs = fin.tile([128, 2], f32)
                 nc.sync.dma_start(out=fs[:], in_=rs_out[:])
                 cnt = fin.tile([128, 1], f32)
                 nc.vector.tensor_scalar_max(out=cnt[:], in0=fs[:, 1:2], scalar1=1.0)
                 rcp = fin.tile([128, 1], f32)
                 nc.vector.reciprocal(out=rcp[:], in_=cnt[:])
                 m = fin.tile([128, 1], f32)
                 nc.vector.tensor_tensor(out=m[:], in0=fs[:, 0:1], in1=rcp[:],
                                         op=mybir.AluOpType.mult)
                 outsb = fin.tile([128, 1], f32)
                 nc.scalar.activation(out=outsb[:], in_=m[:],
                                      func=mybir.ActivationFunctionType.Sigmoid,
                                      bias=brep_sb[:, 0:1], scale=1.0)
                 nc.sync.dma_start(out=d_out[:], in_=outsb[:])

    _split_waits(nc)
    mybir.codegen_inst_isa_subclasses(nc)
    return nc


_NC_CACHE = {}


def _get_nc(reps=1, use_cc=True, mode='full'):
    key = (reps, use_cc, mode)
    if key not in _NC_CACHE:
        _NC_CACHE[key] = _build_nc(reps, use_cc, mode)
    return _NC_CACHE[key]


def _prep_inputs(text, start_times, emb, conv_w, conv_b, W, b):
    bf16 = ml_dtypes.bfloat16
    text = np.asarray(text)[0]              # [N, L]
    st = np.asarray(start_times)[0].astype(np.int64)   # [N]
    emb = np.asarray(emb, dtype=np.float32)
    conv_w = np.asarray(conv_w, dtype=np.float32)
    conv_b = np.asarray(conv_b, dtype=np.float32)
    W = np.asarray(W, dtype=np.float32)
    b = np.asarray(b, dtype=np.float32)

    embp = np.zeros((V, 128), dtype=bf16)
    embp[:, :E] = emb.astype(bf16)

    w01 = np.zeros((128, H), dtype=bf16)
    w01[:64, :] = conv_w[:, :, 0].T.astype(bf16)
    w01[64:, :] = conv_w[:, :, 1].T.astype(bf16)
    w2 = np.ascontiguousarray(conv_w[:, :, 2].T.astype(bf16))
    convb2 = np.ascontiguousarray(conv_b.reshape(2, 128).T.astype(np.float32))

    iota = np.tile(np.arange(S, dtype=np.float32), (128, 1))
    wfeat = np.tile(W[1:H + 1, 0], (128, 1)).astype(np.float32)
    brep = np.full((128, 1), b[0], np.float32)

    delta_g = np.concatenate([[0.0], np.diff(st).astype(np.float32)]).astype(np.float32)
    dw0_g = (delta_g * W[0, 0]).astype(np.float32)

    tok = text.astype(np.int16)             # V=30000 < 2**15
    in_maps = []
    for c in range(NCORES):
        sl = slice(c * NC_NOTES, (c + 1) * NC_NOTES)
        t = tok[sl].reshape(NBLK, NGC, GCHUNK)
        # per-chunk wrap: [GCHUNK//16, 16] -> [16, GCHUNK//16], tiled to 128
        w = t.reshape(NBLK, NGC, GCHUNK // 16, 16)
        w = w.transpose(0, 1, 3, 2)                 # [NBLK, NGC, 16, GCHUNK//16]
        w = np.tile(w, (1, 1, 8, 1))                # [NBLK, NGC, 128, GCHUNK//16]
        idx = np.ascontiguousarray(
            w.transpose(0, 2, 1, 3).reshape(NBLK, 128, BLK_TOK // 16))
        in_maps.append({
            "embp": embp,
            "idx": idx,
            "stf": np.ascontiguousarray(st[sl, None].astype(np.float32)),
            "dw0": np.ascontiguousarray(dw0_g[sl, None]),
            "w01": w01,
            "w2": w2,
            "convb2": convb2,
            "iota": iota,
            "wfeat": wfeat,
            "brep": brep,
        })
    return in_maps


def kernel(**inputs) -> np.ndarray:
    nc = _get_nc()
    in_maps = _prep_inputs(**inputs)
    res = run_bass_kernel_spmd(nc, in_maps, list(range(NCORES))).results
    out = np.concatenate([res[c]["out"] for c in range(NCORES)], axis=0)
    return out.astype(np.float32)


if __name__ == "__main__":
    import jax
    import reference
    cpu = jax.devices("cpu")[0]
    with jax.default_device(cpu):
        ins = {k: np.asarray(v) for k, v in reference.setup_inputs().items()}
        exp = np.asarray(reference.reference(**reference.setup_inputs()))
    got = kernel(**ins)
    err = np.abs(got - exp).max()
    rel = err / max(np.abs(exp).max(), 1e-9)
    print("max abs err:", err, "rel:", rel)


# revision 17
# speedup vs baseline: 148.5121x; 116.5868x over previous
"""Trainium2 Bass kernel for nn_BaseLineModel (segment_reduce).

Model: token-embed -> conv1d(K=3) -> relu -> global max-pool per note,
concat with delta-times, segment-mean over notes sharing a start day,
tiny linear + sigmoid -> [S, 1].

Since the final Linear is applied to the segment-MEAN, and mean/sum
commute with the linear map, we collapse per-note features to a single
scalar z[n] = feats[n]·W[1:] + delta[n]*W[0] on-device, segment-sum
[z, 1] per day ([S, 2] instead of [S, 258]) and finish with
sigmoid(z_s/max(c_s,1) + b) per day on each core.

Sharding: notes are split 8 ways at DAY-ALIGNED cut points (start_times
are globally sorted, so each day's notes land on exactly one core; a
measured ~5ms fixed-cost ReduceScatter is thereby avoided entirely and
the host just selects each day's value from its owning core). Shards are
padded to 2176 notes with zero-contribution pads. Per core: token
embeddings are fetched with non-transpose dma_gather (one 256B
descriptor per token, round-robin over 4 SWDGE queues - measured ~20x
faster than transpose-mode gather), PE-transposed from token-major to
E-major, conv as 2 PSUM-accumulated matmuls per 512 tokens per 128-h
half, DVE max-pool, ACT relu+bias, per-note z via PE transpose + DVE
dot, then a single one-hot matmul per 128-note chunk accumulates
[z, 1] into a [128 day-mod, 8 bk x 2] PSUM tile (one accumulation
group per PSUM bank - interleaved per-column groups corrupt data).
"""

import numpy as np
import ml_dtypes

import concourse.bass as bass
import concourse.mybir as mybir
import concourse.tile as tile
from concourse.bass_utils import run_bass_kernel_spmd
from concourse import library_config
from concourse.masks import make_identity

# ---- problem dims (hardcoded per task contract) ----
N, L, E, H, K, V, S = 16384, 64, 64, 256, 3, 30000, 1024
NCORES = 8
# Day-aligned sharding: cores own disjoint day ranges (start_times are
# globally sorted), so shards are 2048 +- (notes of one boundary day).
# Capacity 2176 with zero-contribution padding (pad notes get bk id 8,
# which matches no bk one-hot).
NC_NOTES = 2176                   # note capacity per core (17 blocks)
NTOK = NC_NOTES * L               # 139264 tokens per core
BLK_NOTES = 128                   # notes per gather block
BLK_TOK = BLK_NOTES * L           # 8192 tokens per block
NBLK = NC_NOTES // BLK_NOTES      # 17
GCHUNK = 512                      # tokens per dma_gather call (desc-ring cap)
NGC = BLK_TOK // GCHUNK           # 16 gather calls per block
NQ = 4                            # SWDGE queues (ucode max)
NCHUNK = NC_NOTES // 128          # 17 note-chunks for segment phase
TMAX = L - K + 1                  # 62 valid conv positions

_SPLIT_MAXW = 1


def _split_waits(nc, maxw=_SPLIT_MAXW):
    """This walrus build rejects >1 sync wait per instruction; move extras
    onto preceding same-engine NOPs (sequencer order preserves semantics)."""
    for bb in nc.main_func.blocks:
        out = []
        for inst in bb.instructions:
            si = inst.sync_info
            waits = list(si.on_wait) if (si is not None and si.on_wait) else []
            if len(waits) > maxw:
                rest = waits[:-maxw]
                si.on_wait = waits[-maxw:]
                for i in range(0, len(rest), maxw):
                    out.append(mybir.InstNoOp(
                        name=f"{inst.name}-wsplit{i}",
                        sync_info=mybir.SyncInfo(on_wait=rest[i:i + maxw], on_update=[]),
                        bass_nofuse=True,
                        engine=inst.engine,
                    ))
            out.append(inst)
        bb.instructions = out


def _build_nc(reps=1, use_cc=True, mode='full'):
    f32 = mybir.dt.float32
    bf16 = mybir.dt.bfloat16
    i16 = mybir.dt.int16

    nc = bass.Bass(num_swdge_queues=NQ)
    d_embp = nc.declare_dram_parameter("embp", [V, 128], bf16, isOutput=False)
    d_idx = nc.declare_dram_parameter("idx", [NBLK, 128, BLK_TOK // 16], i16, isOutput=False)
    d_stm = nc.declare_dram_parameter("stm", [NC_NOTES, 1], f32, isOutput=False)
    d_bki = nc.declare_dram_parameter("bki", [NC_NOTES, 1], f32, isOutput=False)
    d_dw0 = nc.declare_dram_parameter("dw0", [NC_NOTES, 1], f32, isOutput=False)
    d_w01 = nc.declare_dram_parameter("w01", [128, H], bf16, isOutput=False)
    d_w2 = nc.declare_dram_parameter("w2", [64, H], bf16, isOutput=False)
    d_cb = nc.declare_dram_parameter("convb2", [128, 2], f32, isOutput=False)
    d_iot128 = nc.declare_dram_parameter("iot128", [128, 128], f32, isOutput=False)
    d_iot8 = nc.declare_dram_parameter("iot8", [128, 8], f32, isOutput=False)
    d_wfeat = nc.declare_dram_parameter("wfeat", [128, H], f32, isOutput=False)
    d_brep = nc.declare_dram_parameter("brep", [128, 1], f32, isOutput=False)
    d_out = nc.declare_dram_parameter("out", [128, 8], f32, isOutput=True)
    part = None
    if mode == 'partout':
        part = nc.declare_dram_parameter("part", [S, 2], f32, isOutput=True)

    with tile.TileContext(nc) as tc:
        nc.gpsimd.load_library(library_config.mlp)
        nidx_reg_cm = nc.gpsimd.register("nidx")
        nidx_reg = nidx_reg_cm.__enter__()
        nc.gpsimd.reg_mov(nidx_reg, GCHUNK)
        with (
            tc.tile_pool(name="cst", bufs=1) as cp,
            tc.tile_pool(name="feat", bufs=1) as fp,
        ):
         for _rep in range(reps):
             w01_sb = cp.tile([128, H], bf16)
             w2_sb = cp.tile([64, H], bf16)
             cb_sb = cp.tile([128, 2], f32)
             identb = cp.tile([128, 128], bf16)
             identf = cp.tile([128, 128], f32)
             nc.sync.dma_start(out=w01_sb[:], in_=d_w01[:])
             nc.sync.dma_start(out=w2_sb[:], in_=d_w2[:])
             nc.sync.dma_start(out=cb_sb[:], in_=d_cb[:])
             make_identity(nc, identb[:])
             make_identity(nc, identf[:])
             feats = [fp.tile([128, NC_NOTES], f32, name=f"feats{hh}") for hh in range(2)]

             # ---- P1: gather (token-major) + PE transpose + conv + maxpool ----
             with (
                 tc.tile_pool(name="gath", bufs=3) as gp,
                 tc.tile_pool(name="xep", bufs=2) as xp,
                 tc.tile_pool(name="tpsum", bufs=2, space="PSUM") as tp,
                 tc.tile_pool(name="ypsum", bufs=4, space="PSUM") as yp,
             ):
                 for b in range(NBLK):
                     idx_sb = gp.tile([128, BLK_TOK // 16], i16, tag="idx")
                     nc.sync.dma_start(out=idx_sb[:], in_=d_idx[b])
                     stg = gp.tile([128, BLK_TOK], bf16, tag="stg")
                     for c in range(NGC):
                         nc.gpsimd.dma_gather(
                             out_ap=stg[:, c * GCHUNK:(c + 1) * GCHUNK]
                                 .rearrange("p (g e) -> p g e", e=128),
                             in_ap=d_embp[:],
                             idxs_ap=idx_sb[:, c * (GCHUNK // 16):(c + 1) * (GCHUNK // 16)],
                             num_idxs=GCHUNK,
                             num_idxs_reg=nidx_reg,
                             elem_size=128,
                             transpose=False,
                             queue_num=c % NQ,
                         )
                     xE = xp.tile([128, BLK_TOK], bf16, tag="xe")
                     if mode != 'gather':
                         # token-major [128 tok, 128 e] tiles -> E-major xE
                         for q in range(BLK_TOK // 512):
                             tq = tp.tile([128, 512], bf16, tag="t", name=f"t{b}_{q}")
                             for j in range(4):
                                 g = q * 4 + j
                                 nc.tensor.transpose(
                                     out=tq[:, j * 128:(j + 1) * 128],
                                     in_=stg[:, g * 128:(g + 1) * 128],
                                     identity=identb[:])
                             nc.scalar.activation(
                                 out=xE[0:64, q * 512:(q + 1) * 512],
                                 in_=tq[0:64, :],
                                 func=mybir.ActivationFunctionType.Copy,
                                 scale=1.0)
                         # stack k=1 shift into partitions 64:128
                         nc.sync.dma_start(out=xE[64:128, 0:BLK_TOK - 1],
                                           in_=xE[0:64, 1:BLK_TOK])
                         for g in range(16):
                             c0 = g * 512
                             for hh in range(2):
                                 y_ps = yp.tile([128, 512], f32, tag="y", name=f"y{b}_{g}_{hh}")
                                 nc.tensor.matmul(out=y_ps[:],
                                                  lhsT=w01_sb[:, hh * 128:(hh + 1) * 128],
                                                  rhs=xE[:, c0:c0 + 512],
                                                  start=True, stop=False)
                                 nc.tensor.matmul(out=y_ps[:, 0:510],
                                                  lhsT=w2_sb[:, hh * 128:(hh + 1) * 128],
                                                  rhs=xE[0:64, c0 + 2:c0 + 512],
                                                  start=False, stop=True)
                                 nc.vector.reduce_max(
                                     out=feats[hh][:, b * BLK_NOTES + g * 8:
                                                   b * BLK_NOTES + g * 8 + 8],
                                     in_=y_ps[:].rearrange("p (n l) -> p n l", l=L)[:, :, 0:TMAX],
                                     axis=mybir.AxisListType.X)

             # ---- P2: relu(feats + conv_b) ----
             if mode == 'gather':
                 nc.vector.memset(feats[0][:], 0.0)
                 nc.vector.memset(feats[1][:], 0.0)
             for hh in range(2):
                 nc.scalar.activation(out=feats[hh][:], in_=feats[hh][:],
                                      func=mybir.ActivationFunctionType.Relu,
                                      bias=cb_sb[:, hh:hh + 1], scale=1.0)

             # ---- P3: per-note z = feats.W[1:] + delta*W[0]; day one-hot segsum ----
             wfeat_sb = cp.tile([128, H], f32)
             nc.sync.dma_start(out=wfeat_sb[:], in_=d_wfeat[:])
             iot128_sb = cp.tile([128, 128], f32)
             nc.sync.dma_start(out=iot128_sb[:], in_=d_iot128[:])
             iot8_sb = cp.tile([128, 8], f32)
             nc.sync.dma_start(out=iot8_sb[:], in_=d_iot8[:])
             with (
                 tc.tile_pool(name="segsb", bufs=2) as ssp,
                 tc.tile_pool(name="ftps", bufs=2, space="PSUM") as ftp,
                 tc.tile_pool(name="segps", bufs=1, space="PSUM") as pp,
             ):
                 seg_ps = pp.tile([128, 16], f32, name="seg")
                 for i in range(NCHUNK):
                     ft = ftp.tile([128, H], f32, tag="ft", name=f"ft{i}")
                     for hh in range(2):
                         nc.tensor.transpose(
                             out=ft[:, hh * 128:(hh + 1) * 128],
                             in_=feats[hh][:, i * 128:(i + 1) * 128],
                             identity=identf[:])
                     prod = ssp.tile([128, H], f32, tag="prod")
                     nc.vector.tensor_tensor(out=prod[:], in0=ft[:], in1=wfeat_sb[:],
                                             op=mybir.AluOpType.mult)
                     dw0 = ssp.tile([128, 1], f32, tag="dw0")
                     nc.sync.dma_start(out=dw0[:], in_=d_dw0[i * 128:(i + 1) * 128, :])
                     z2 = ssp.tile([128, 1], f32, tag="z2")
                     nc.vector.reduce_sum(out=z2[:], in_=prod[:],
                                          axis=mybir.AxisListType.X)
                     nc.vector.tensor_add(out=z2[:], in0=z2[:], in1=dw0[:])
                     st_sb = ssp.tile([128, 2], f32, tag="st")
                     nc.sync.dma_start(out=st_sb[:, 0:1], in_=d_stm[i * 128:(i + 1) * 128, :])
                     nc.sync.dma_start(out=st_sb[:, 1:2], in_=d_bki[i * 128:(i + 1) * 128, :])
                     # one-hot of day mod 128 (lhsT) and day//128 (bk mask)
                     ohm = ssp.tile([128, 128], f32, tag="ohm")
                     nc.vector.tensor_tensor(out=ohm[:],
                                             in0=st_sb[:, 0:1].to_broadcast([128, 128]),
                                             in1=iot128_sb[:],
                                             op=mybir.AluOpType.is_equal)
                     bkoh = ssp.tile([128, 8], f32, tag="bkoh")
                     nc.vector.tensor_tensor(out=bkoh[:],
                                             in0=st_sb[:, 1:2].to_broadcast([128, 8]),
                                             in1=iot8_sb[:],
                                             op=mybir.AluOpType.is_equal)
                     # rhs16[n, bk*2+0] = z[n]*bkmask ; rhs16[n, bk*2+1] = bkmask
                     rhs16 = ssp.tile([128, 16], f32, tag="rhs16")
                     rvc = rhs16[:].rearrange("p (g c) -> p c g", c=2)
                     nc.vector.tensor_tensor(out=rvc[:, 0, :], in0=bkoh[:],
                                             in1=z2[:, 0:1].to_broadcast([128, 8]),
                                             op=mybir.AluOpType.mult)
                     nc.vector.tensor_copy(out=rvc[:, 1, :], in_=bkoh[:])
                     nc.tensor.matmul(out=seg_ps[:], lhsT=ohm[:], rhs=rhs16[:],
                                      start=(i == 0), stop=(i == NCHUNK - 1))
                 seg_sb = ssp.tile([128, 16], f32, tag="segout")
                 nc.vector.tensor_copy(out=seg_sb[:], in_=seg_ps[:])
                 if mode == 'partout':
                     for bk in range(8):
                         nc.sync.dma_start(out=part[bk * 128:(bk + 1) * 128, :],
                                           in_=seg_sb[:, bk * 2:bk * 2 + 2])

                 # ---- P4: per-core finalize (day-aligned shards: no collective;
                 # out[p, bk] = sigmoid(z/max(c,1) + b) for day bk*128+p) ----
                 brep_sb = ssp.tile([128, 1], f32, tag="brep")
                 nc.sync.dma_start(out=brep_sb[:], in_=d_brep[:])
                 sv = seg_sb[:].rearrange("p (g c) -> p c g", c=2)
                 cnt8 = ssp.tile([128, 8], f32, tag="cnt8")
                 nc.vector.tensor_scalar_max(out=cnt8[:], in0=sv[:, 1, :], scalar1=1.0)
                 rcp8 = ssp.tile([128, 8], f32, tag="rcp8")
                 nc.vector.reciprocal(out=rcp8[:], in_=cnt8[:])
                 m8 = ssp.tile([128, 8], f32, tag="m8")
                 nc.vector.tensor_tensor(out=m8[:], in0=sv[:, 0, :], in1=rcp8[:],
                                         op=mybir.AluOpType.mult)
                 out8 = ssp.tile([128, 8], f32, tag="out8")
                 nc.scalar.activation(out=out8[:], in_=m8[:],
                                      func=mybir.ActivationFunctionType.Sigmoid,
                                      bias=brep_sb[:, 0:1], scale=1.0)
                 nc.sync.dma_start(out=d_out[:], in_=out8[:])

    _split_waits(nc)
    mybir.codegen_inst_isa_subclasses(nc)
    return nc


_NC_CACHE = {}


def _get_nc(reps=1, use_cc=True, mode='full'):
    key = (reps, use_cc, mode)
    if key not in _NC_CACHE:
        _NC_CACHE[key] = _build_nc(reps, use_cc, mode)
    return _NC_CACHE[key]


def _day_cuts(st):
    """Day-aligned note cut points: cut c is the first note of the day
    containing note c*(N//NCORES), so no day spans two shards."""
    cuts = [0]
    for c in range(1, NCORES):
        cuts.append(int(np.searchsorted(st, st[c * (N // NCORES)], side="left")))
    cuts.append(N)
    return cuts


def _prep_inputs(text, start_times, emb, conv_w, conv_b, W, b):
    bf16 = ml_dtypes.bfloat16
    text = np.asarray(text)[0]              # [N, L]
    st = np.asarray(start_times)[0].astype(np.int64)   # [N]
    emb = np.asarray(emb, dtype=np.float32)
    conv_w = np.asarray(conv_w, dtype=np.float32)
    conv_b = np.asarray(conv_b, dtype=np.float32)
    W = np.asarray(W, dtype=np.float32)
    b = np.asarray(b, dtype=np.float32)

    embp = np.zeros((V, 128), dtype=bf16)
    embp[:, :E] = emb.astype(bf16)

    w01 = np.zeros((128, H), dtype=bf16)
    w01[:64, :] = conv_w[:, :, 0].T.astype(bf16)
    w01[64:, :] = conv_w[:, :, 1].T.astype(bf16)
    w2 = np.ascontiguousarray(conv_w[:, :, 2].T.astype(bf16))
    convb2 = np.ascontiguousarray(conv_b.reshape(2, 128).T.astype(np.float32))

    iot128 = np.tile(np.arange(128, dtype=np.float32), (128, 1))
    iot8 = np.tile(np.arange(8, dtype=np.float32), (128, 1))
    wfeat = np.tile(W[1:H + 1, 0], (128, 1)).astype(np.float32)
    brep = np.full((128, 1), b[0], np.float32)

    delta_g = np.concatenate([[0.0], np.diff(st).astype(np.float32)]).astype(np.float32)
    dw0_g = (delta_g * W[0, 0]).astype(np.float32)

    tok = text.astype(np.int16)             # V=30000 < 2**15
    cuts = _day_cuts(st)
    in_maps = []
    for c in range(NCORES):
        lo, hi = cuts[c], cuts[c + 1]
        nre = hi - lo
        assert 0 < nre <= NC_NOTES, (c, nre)
        tok_c = np.zeros((NC_NOTES, L), np.int16)
        tok_c[:nre] = tok[lo:hi]
        stm_c = np.zeros((NC_NOTES, 1), np.float32)
        stm_c[:nre, 0] = (st[lo:hi] % 128).astype(np.float32)
        bki_c = np.full((NC_NOTES, 1), 8.0, np.float32)   # pad: matches no bk
        bki_c[:nre, 0] = (st[lo:hi] // 128).astype(np.float32)
        dw0_c = np.zeros((NC_NOTES, 1), np.float32)
        dw0_c[:nre, 0] = dw0_g[lo:hi]

        t = tok_c.reshape(NBLK, NGC, GCHUNK)
        # per-chunk wrap: [GCHUNK//16, 16] -> [16, GCHUNK//16], tiled to 128
        w = t.reshape(NBLK, NGC, GCHUNK // 16, 16)
        w = w.transpose(0, 1, 3, 2)                 # [NBLK, NGC, 16, GCHUNK//16]
        w = np.tile(w, (1, 1, 8, 1))                # [NBLK, NGC, 128, GCHUNK//16]
        idx = np.ascontiguousarray(
            w.transpose(0, 2, 1, 3).reshape(NBLK, 128, BLK_TOK // 16))
        in_maps.append({
            "embp": embp,
            "idx": idx,
            "stm": stm_c,
            "bki": bki_c,
            "dw0": dw0_c,
            "w01": w01,
            "w2": w2,
            "convb2": convb2,
            "iot128": iot128,
            "iot8": iot8,
            "wfeat": wfeat,
            "brep": brep,
        })
    return in_maps


def kernel(**inputs) -> np.ndarray:
    nc = _get_nc()
    in_maps = _prep_inputs(**inputs)
    res = run_bass_kernel_spmd(nc, in_maps, list(range(NCORES))).results

    st = np.asarray(inputs["start_times"])[0].astype(np.int64)
    cuts = _day_cuts(st)
    owner = np.zeros(S, np.int64)
    for c in range(NCORES):
        owner[st[cuts[c]]:st[cuts[c + 1] - 1] + 1] = c
    outs = np.stack([res[c]["out"] for c in range(NCORES)])   # [8, 128, 8]
    days = np.arange(S)
    out = outs[owner, days % 128, days // 128].astype(np.float32)
    return out[:, None]


if __name__ == "__main__":
    import jax
    import reference
    cpu = jax.devices("cpu")[0]
    with jax.default_device(cpu):
        ins = {k: np.asarray(v) for k, v in reference.setup_inputs().items()}
        exp = np.asarray(reference.reference(**reference.setup_inputs()))
    got = kernel(**ins)
    err = np.abs(got - exp).max()
    rel = err / max(np.abs(exp).max(), 1e-9)
    print("max abs err:", err, "rel:", rel)


# revision 18
# speedup vs baseline: 161.7571x; 1.0892x over previous
"""Trainium2 Bass kernel for nn_BaseLineModel (segment_reduce).

Model: token-embed -> conv1d(K=3) -> relu -> global max-pool per note,
concat with delta-times, segment-mean over notes sharing a start day,
tiny linear + sigmoid -> [S, 1].

Since the final Linear is applied to the segment-MEAN, and mean/sum
commute with the linear map, we collapse per-note features to a single
scalar z[n] = feats[n]·W[1:] + delta[n]*W[0] on-device, segment-sum
[z, 1] per day ([S, 2] instead of [S, 258]) and finish with
sigmoid(z_s/max(c_s,1) + b) per day on each core.

Sharding: notes are split 8 ways at DAY-ALIGNED cut points (start_times
are globally sorted, so each day's notes land on exactly one core; a
measured ~5ms fixed-cost ReduceScatter is thereby avoided entirely and
the host just selects each day's value from its owning core). Shards are
padded to 2176 notes with zero-contribution pads. Per core: token
embeddings are fetched with non-transpose dma_gather (one 256B
descriptor per token, round-robin over 4 SWDGE queues - measured ~20x
faster than transpose-mode gather), PE-transposed from token-major to
E-major, conv as 2 PSUM-accumulated matmuls per 512 tokens per 128-h
half, DVE max-pool, ACT relu+bias, per-note z via PE transpose + DVE
dot, then a single one-hot matmul per 128-note chunk accumulates
[z, 1] into a [128 day-mod, 8 bk x 2] PSUM tile (one accumulation
group per PSUM bank - interleaved per-column groups corrupt data).
"""

import numpy as np
import ml_dtypes

import concourse.bass as bass
import concourse.mybir as mybir
import concourse.tile as tile
from concourse.bass_utils import run_bass_kernel_spmd
from concourse import library_config
from concourse.masks import make_identity

# ---- problem dims (hardcoded per task contract) ----
N, L, E, H, K, V, S = 16384, 64, 64, 256, 3, 30000, 1024
NCORES = 8
# Day-aligned sharding: cores own disjoint day ranges (start_times are
# globally sorted), so shards are 2048 +- (notes of one boundary day).
# Capacity 2176 with zero-contribution padding (pad notes get bk id 8,
# which matches no bk one-hot).
NC_NOTES = 2176                   # note capacity per core (17 blocks)
NTOK = NC_NOTES * L               # 139264 tokens per core
BLK_NOTES = 128                   # notes per gather block
BLK_TOK = BLK_NOTES * L           # 8192 tokens per block
NBLK = NC_NOTES // BLK_NOTES      # 17
GCHUNK = 1024                     # tokens per dma_gather call (desc-ring cap)
NGC = BLK_TOK // GCHUNK           # 16 gather calls per block
NQ = 4                            # SWDGE queues (ucode max)
NCHUNK = NC_NOTES // 128          # 17 note-chunks for segment phase
TMAX = L - K + 1                  # 62 valid conv positions

_SPLIT_MAXW = 1


def _split_waits(nc, maxw=_SPLIT_MAXW):
    """This walrus build rejects >1 sync wait per instruction; move extras
    onto preceding same-engine NOPs (sequencer order preserves semantics)."""
    for bb in nc.main_func.blocks:
        out = []
        for inst in bb.instructions:
            si = inst.sync_info
            waits = list(si.on_wait) if (si is not None and si.on_wait) else []
            if len(waits) > maxw:
                rest = waits[:-maxw]
                si.on_wait = waits[-maxw:]
                for i in range(0, len(rest), maxw):
                    out.append(mybir.InstNoOp(
                        name=f"{inst.name}-wsplit{i}",
                        sync_info=mybir.SyncInfo(on_wait=rest[i:i + maxw], on_update=[]),
                        bass_nofuse=True,
                        engine=inst.engine,
                    ))
            out.append(inst)
        bb.instructions = out


def _build_nc(reps=1, use_cc=True, mode='full'):
    f32 = mybir.dt.float32
    bf16 = mybir.dt.bfloat16
    i16 = mybir.dt.int16

    nc = bass.Bass(num_swdge_queues=NQ)
    d_embp = nc.declare_dram_parameter("embp", [V, 128], bf16, isOutput=False)
    d_idx = nc.declare_dram_parameter("idx", [NBLK, 128, BLK_TOK // 16], i16, isOutput=False)
    d_stm = nc.declare_dram_parameter("stm", [NC_NOTES, 1], f32, isOutput=False)
    d_bki = nc.declare_dram_parameter("bki", [NC_NOTES, 1], f32, isOutput=False)
    d_dw0 = nc.declare_dram_parameter("dw0", [NC_NOTES, 1], f32, isOutput=False)
    d_w01 = nc.declare_dram_parameter("w01", [128, H], bf16, isOutput=False)
    d_w2 = nc.declare_dram_parameter("w2", [64, H], bf16, isOutput=False)
    d_cb = nc.declare_dram_parameter("convb2", [128, 2], f32, isOutput=False)
    d_iot128 = nc.declare_dram_parameter("iot128", [128, 128], f32, isOutput=False)
    d_iot8 = nc.declare_dram_parameter("iot8", [128, 8], f32, isOutput=False)
    d_wfeat = nc.declare_dram_parameter("wfeat", [128, H], f32, isOutput=False)
    d_brep = nc.declare_dram_parameter("brep", [128, 1], f32, isOutput=False)
    d_out = nc.declare_dram_parameter("out", [128, 8], f32, isOutput=True)
    part = None
    if mode == 'partout':
        part = nc.declare_dram_parameter("part", [S, 2], f32, isOutput=True)

    with tile.TileContext(nc) as tc:
        nc.gpsimd.load_library(library_config.mlp)
        nidx_reg_cm = nc.gpsimd.register("nidx")
        nidx_reg = nidx_reg_cm.__enter__()
        nc.gpsimd.reg_mov(nidx_reg, GCHUNK)
        with (
            tc.tile_pool(name="cst", bufs=1) as cp,
            tc.tile_pool(name="feat", bufs=1) as fp,
        ):
         for _rep in range(reps):
             w01_sb = cp.tile([128, H], bf16)
             w2_sb = cp.tile([64, H], bf16)
             cb_sb = cp.tile([128, 2], f32)
             identb = cp.tile([128, 128], bf16)
             identf = cp.tile([128, 128], f32)
             nc.sync.dma_start(out=w01_sb[:], in_=d_w01[:])
             nc.sync.dma_start(out=w2_sb[:], in_=d_w2[:])
             nc.sync.dma_start(out=cb_sb[:], in_=d_cb[:])
             make_identity(nc, identb[:])
             make_identity(nc, identf[:])
             feats = [fp.tile([128, NC_NOTES], f32, name=f"feats{hh}") for hh in range(2)]

             # ---- P1: gather (token-major) + PE transpose + conv + maxpool ----
             with (
                 tc.tile_pool(name="gath", bufs=3) as gp,
                 tc.tile_pool(name="xep", bufs=2) as xp,
                 tc.tile_pool(name="tpsum", bufs=2, space="PSUM") as tp,
                 tc.tile_pool(name="ypsum", bufs=4, space="PSUM") as yp,
             ):
                 for b in range(NBLK):
                     idx_sb = gp.tile([128, BLK_TOK // 16], i16, tag="idx")
                     nc.sync.dma_start(out=idx_sb[:], in_=d_idx[b])
                     stg = gp.tile([128, BLK_TOK], bf16, tag="stg")
                     for c in range(NGC):
                         nc.gpsimd.dma_gather(
                             out_ap=stg[:, c * GCHUNK:(c + 1) * GCHUNK]
                                 .rearrange("p (g e) -> p g e", e=128),
                             in_ap=d_embp[:],
                             idxs_ap=idx_sb[:, c * (GCHUNK // 16):(c + 1) * (GCHUNK // 16)],
                             num_idxs=GCHUNK,
                             num_idxs_reg=nidx_reg,
                             elem_size=128,
                             transpose=False,
                             queue_num=c % NQ,
                         )
                     xE = xp.tile([128, BLK_TOK], bf16, tag="xe")
                     if mode != 'gather':
                         # token-major [128 tok, 128 e] tiles -> E-major xE
                         for q in range(BLK_TOK // 512):
                             tq = tp.tile([128, 512], bf16, tag="t", name=f"t{b}_{q}")
                             for j in range(4):
                                 g = q * 4 + j
                                 nc.tensor.transpose(
                                     out=tq[:, j * 128:(j + 1) * 128],
                                     in_=stg[:, g * 128:(g + 1) * 128],
                                     identity=identb[:])
                             nc.scalar.activation(
                                 out=xE[0:64, q * 512:(q + 1) * 512],
                                 in_=tq[0:64, :],
                                 func=mybir.ActivationFunctionType.Copy,
                                 scale=1.0)
                         # stack k=1 shift into partitions 64:128
                         nc.sync.dma_start(out=xE[64:128, 0:BLK_TOK - 1],
                                           in_=xE[0:64, 1:BLK_TOK])
                         for g in range(16):
                             c0 = g * 512
                             for hh in range(2):
                                 y_ps = yp.tile([128, 512], f32, tag="y", name=f"y{b}_{g}_{hh}")
                                 nc.tensor.matmul(out=y_ps[:],
                                                  lhsT=w01_sb[:, hh * 128:(hh + 1) * 128],
                                                  rhs=xE[:, c0:c0 + 512],
                                                  start=True, stop=False)
                                 nc.tensor.matmul(out=y_ps[:, 0:510],
                                                  lhsT=w2_sb[:, hh * 128:(hh + 1) * 128],
                                                  rhs=xE[0:64, c0 + 2:c0 + 512],
                                                  start=False, stop=True)
                                 nc.vector.reduce_max(
                                     out=feats[hh][:, b * BLK_NOTES + g * 8:
                                                   b * BLK_NOTES + g * 8 + 8],
                                     in_=y_ps[:].rearrange("p (n l) -> p n l", l=L)[:, :, 0:TMAX],
                                     axis=mybir.AxisListType.X)

             # ---- P2: relu(feats + conv_b) ----
             if mode == 'gather':
                 nc.vector.memset(feats[0][:], 0.0)
                 nc.vector.memset(feats[1][:], 0.0)
             for hh in range(2):
                 nc.scalar.activation(out=feats[hh][:], in_=feats[hh][:],
                                      func=mybir.ActivationFunctionType.Relu,
                                      bias=cb_sb[:, hh:hh + 1], scale=1.0)

             # ---- P3: per-note z = feats.W[1:] + delta*W[0]; day one-hot segsum ----
             wfeat_sb = cp.tile([128, H], f32)
             nc.sync.dma_start(out=wfeat_sb[:], in_=d_wfeat[:])
             iot128_sb = cp.tile([128, 128], f32)
             nc.sync.dma_start(out=iot128_sb[:], in_=d_iot128[:])
             iot8_sb = cp.tile([128, 8], f32)
             nc.sync.dma_start(out=iot8_sb[:], in_=d_iot8[:])
             with (
                 tc.tile_pool(name="segsb", bufs=2) as ssp,
                 tc.tile_pool(name="ftps", bufs=2, space="PSUM") as ftp,
                 tc.tile_pool(name="segps", bufs=1, space="PSUM") as pp,
             ):
                 seg_ps = pp.tile([128, 16], f32, name="seg")
                 for i in range(NCHUNK):
                     ft = ftp.tile([128, H], f32, tag="ft", name=f"ft{i}")
                     for hh in range(2):
                         nc.tensor.transpose(
                             out=ft[:, hh * 128:(hh + 1) * 128],
                             in_=feats[hh][:, i * 128:(i + 1) * 128],
                             identity=identf[:])
                     prod = ssp.tile([128, H], f32, tag="prod")
                     nc.vector.tensor_tensor(out=prod[:], in0=ft[:], in1=wfeat_sb[:],
                                             op=mybir.AluOpType.mult)
                     dw0 = ssp.tile([128, 1], f32, tag="dw0")
                     nc.sync.dma_start(out=dw0[:], in_=d_dw0[i * 128:(i + 1) * 128, :])
                     z2 = ssp.tile([128, 1], f32, tag="z2")
                     nc.vector.reduce_sum(out=z2[:], in_=prod[:],
                                          axis=mybir.AxisListType.X)
                     nc.vector.tensor_add(out=z2[:], in0=z2[:], in1=dw0[:])
                     st_sb = ssp.tile([128, 2], f32, tag="st")
                     nc.sync.dma_start(out=st_sb[:, 0:1], in_=d_stm[i * 128:(i + 1) * 128, :])
                     nc.sync.dma_start(out=st_sb[:, 1:2], in_=d_bki[i * 128:(i + 1) * 128, :])
                     # one-hot of day mod 128 (lhsT) and day//128 (bk mask)
                     ohm = ssp.tile([128, 128], f32, tag="ohm")
                     nc.vector.tensor_tensor(out=ohm[:],
                                             in0=st_sb[:, 0:1].to_broadcast([128, 128]),
                                             in1=iot128_sb[:],
                                             op=mybir.AluOpType.is_equal)
                     bkoh = ssp.tile([128, 8], f32, tag="bkoh")
                     nc.vector.tensor_tensor(out=bkoh[:],
                                             in0=st_sb[:, 1:2].to_broadcast([128, 8]),
                                             in1=iot8_sb[:],
                                             op=mybir.AluOpType.is_equal)
                     # rhs16[n, bk*2+0] = z[n]*bkmask ; rhs16[n, bk*2+1] = bkmask
                     rhs16 = ssp.tile([128, 16], f32, tag="rhs16")
                     rvc = rhs16[:].rearrange("p (g c) -> p c g", c=2)
                     nc.vector.tensor_tensor(out=rvc[:, 0, :], in0=bkoh[:],
                                             in1=z2[:, 0:1].to_broadcast([128, 8]),
                                             op=mybir.AluOpType.mult)
                     nc.vector.tensor_copy(out=rvc[:, 1, :], in_=bkoh[:])
                     nc.tensor.matmul(out=seg_ps[:], lhsT=ohm[:], rhs=rhs16[:],
                                      start=(i == 0), stop=(i == NCHUNK - 1))
                 seg_sb = ssp.tile([128, 16], f32, tag="segout")
                 nc.vector.tensor_copy(out=seg_sb[:], in_=seg_ps[:])
                 if mode == 'partout':
                     for bk in range(8):
                         nc.sync.dma_start(out=part[bk * 128:(bk + 1) * 128, :],
                                           in_=seg_sb[:, bk * 2:bk * 2 + 2])

                 # ---- P4: per-core finalize (day-aligned shards: no collective;
                 # out[p, bk] = sigmoid(z/max(c,1) + b) for day bk*128+p) ----
                 brep_sb = ssp.tile([128, 1], f32, tag="brep")
                 nc.sync.dma_start(out=brep_sb[:], in_=d_brep[:])
                 sv = seg_sb[:].rearrange("p (g c) -> p c g", c=2)
                 cnt8 = ssp.tile([128, 8], f32, tag="cnt8")
                 nc.vector.tensor_scalar_max(out=cnt8[:], in0=sv[:, 1, :], scalar1=1.0)
                 rcp8 = ssp.tile([128, 8], f32, tag="rcp8")
                 nc.vector.reciprocal(out=rcp8[:], in_=cnt8[:])
                 m8 = ssp.tile([128, 8], f32, tag="m8")
                 nc.vector.tensor_tensor(out=m8[:], in0=sv[:, 0, :], in1=rcp8[:],
                                         op=mybir.AluOpType.mult)
                 out8 = ssp.tile([128, 8], f32, tag="out8")
                 nc.scalar.activation(out=out8[:], in_=m8[:],
                                      func=mybir.ActivationFunctionType.Sigmoid,
                                      bias=brep_sb[:, 0:1], scale=1.0)
                 nc.sync.dma_start(out=d_out[:], in_=out8[:])

    _split_waits(nc)
    mybir.codegen_inst_isa_subclasses(nc)
    return nc


_NC_CACHE = {}


def _get_nc(reps=1, use_cc=True, mode='full'):
    key = (reps, use_cc, mode)
    if key not in _NC_CACHE:
        _NC_CACHE[key] = _build_nc(reps, use_cc, mode)
    return _NC_CACHE[key]


def _day_cuts(st):
    """Day-aligned note cut points: cut c is the first note of the day
    containing note c*(N//NCORES), so no day spans two shards."""
    cuts = [0]
    for c in range(1, NCORES):
        cuts.append(int(np.searchsorted(st, st[c * (N // NCORES)], side="left")))
    cuts.append(N)
    return cuts


def _prep_inputs(text, start_times, emb, conv_w, conv_b, W, b):
    bf16 = ml_dtypes.bfloat16
    text = np.asarray(text)[0]              # [N, L]
    st = np.asarray(start_times)[0].astype(np.int64)   # [N]
    emb = np.asarray(emb, dtype=np.float32)
    conv_w = np.asarray(conv_w, dtype=np.float32)
    conv_b = np.asarray(conv_b, dtype=np.float32)
    W = np.asarray(W, dtype=np.float32)
    b = np.asarray(b, dtype=np.float32)

    embp = np.zeros((V, 128), dtype=bf16)
    embp[:, :E] = emb.astype(bf16)

    w01 = np.zeros((128, H), dtype=bf16)
    w01[:64, :] = conv_w[:, :, 0].T.astype(bf16)
    w01[64:, :] = conv_w[:, :, 1].T.astype(bf16)
    w2 = np.ascontiguousarray(conv_w[:, :, 2].T.astype(bf16))
    convb2 = np.ascontiguousarray(conv_b.reshape(2, 128).T.astype(np.float32))

    iot128 = np.tile(np.arange(128, dtype=np.float32), (128, 1))
    iot8 = np.tile(np.arange(8, dtype=np.float32), (128, 1))
    wfeat = np.tile(W[1:H + 1, 0], (128, 1)).astype(np.float32)
    brep = np.full((128, 1), b[0], np.float32)

    delta_g = np.concatenate([[0.0], np.diff(st).astype(np.float32)]).astype(np.float32)
    dw0_g = (delta_g * W[0, 0]).astype(np.float32)

    tok = text.astype(np.int16)             # V=30000 < 2**15
    cuts = _day_cuts(st)
    in_maps = []
    for c in range(NCORES):
        lo, hi = cuts[c], cuts[c + 1]
        nre = hi - lo
        assert 0 < nre <= NC_NOTES, (c, nre)
        tok_c = np.zeros((NC_NOTES, L), np.int16)
        tok_c[:nre] = tok[lo:hi]
        stm_c = np.zeros((NC_NOTES, 1), np.float32)
        stm_c[:nre, 0] = (st[lo:hi] % 128).astype(np.float32)
        bki_c = np.full((NC_NOTES, 1), 8.0, np.float32)   # pad: matches no bk
        bki_c[:nre, 0] = (st[lo:hi] // 128).astype(np.float32)
        dw0_c = np.zeros((NC_NOTES, 1), np.float32)
        dw0_c[:nre, 0] = dw0_g[lo:hi]

        t = tok_c.reshape(NBLK, NGC, GCHUNK)
        # per-chunk wrap: [GCHUNK//16, 16] -> [16, GCHUNK//16], tiled to 128
        w = t.reshape(NBLK, NGC, GCHUNK // 16, 16)
        w = w.transpose(0, 1, 3, 2)                 # [NBLK, NGC, 16, GCHUNK//16]
        w = np.tile(w, (1, 1, 8, 1))                # [NBLK, NGC, 128, GCHUNK//16]
        idx = np.ascontiguousarray(
            w.transpose(0, 2, 1, 3).reshape(NBLK, 128, BLK_TOK // 16))
        in_maps.append({
            "embp": embp,
            "idx": idx,
            "stm": stm_c,
            "bki": bki_c,
            "dw0": dw0_c,
            "w01": w01,
            "w2": w2,
            "convb2": convb2,
            "iot128": iot128,
            "iot8": iot8,
            "wfeat": wfeat,
            "brep": brep,
        })
    return in_maps


def kernel(**inputs) -> np.ndarray:
    nc = _get_nc()
    in_maps = _prep_inputs(**inputs)
    res = run_bass_kernel_spmd(nc, in_maps, list(range(NCORES))).results

    st = np.asarray(inputs["start_times"])[0].astype(np.int64)
    cuts = _day_cuts(st)
    owner = np.zeros(S, np.int64)
    for c in range(NCORES):
        owner[st[cuts[c]]:st[cuts[c + 1] - 1] + 1] = c
    outs = np.stack([res[c]["out"] for c in range(NCORES)])   # [8, 128, 8]
    days = np.arange(S)
    out = outs[owner, days % 128, days // 128].astype(np.float32)
    return out[:, None]


if __name__ == "__main__":
    import jax
    import reference
    cpu = jax.devices("cpu")[0]
    with jax.default_device(cpu):
        ins = {k: np.asarray(v) for k, v in reference.setup_inputs().items()}
        exp = np.asarray(reference.reference(**reference.setup_inputs()))
    got = kernel(**ins)
    err = np.abs(got - exp).max()
    rel = err / max(np.abs(exp).max(), 1e-9)
    print("max abs err:", err, "rel:", rel)
